# revision 1
# baseline (speedup 1.0000x reference)
"""GAT layer on 8 Trainium2 NeuronCores (Bass/Tile).

Strategy (target-per-partition layout, edge gathers via ANT dma_gather):
  - Targets sharded across 8 cores (12500 each). Per core, targets are
    reordered (lex4 clustering by per-chunk source counts) into 98 blocks
    of 128; block b target v lives on SBUF partition v.
  - Phase B builds a DRAM table row per node: [p bf16(128) | alpha_src
    f32(8) | junk] = 512B rows, via PE matmuls (stationary = xT tile,
    moving = [W_proj | W_proj @ Ablk]).
  - Per block, per source-chunk (4 chunks of <=25088 nodes so indices fit
    int16), dma_gather pulls each edge slot's row into [128 targets, W,
    256] bf16 tiles. Padding slots point at a per-chunk sentinel row
    (p=0, alpha=-300 -> exp(s)~0).
  - s = alpha + beta[target] (beta broadcast per partition), Lrelu(0.2),
    Exp -> E. Factored softmax: U = sum_d E*p, D = sum_d E, out =
    U/D + skip, then ELU. beta/skip come from a per-block matmul with
    stationary = xTperm tile, moving = [W_skip | W_proj @ Bblk].
"""

import os
import sys

sys.path.insert(0, "/opt/trn_rl_repo")

import numpy as np
from contextlib import ExitStack

import concourse.bass as bass
import concourse.bacc as bacc
import concourse.tile as tile
from concourse import mybir
from concourse._compat import cdiv
from concourse.bass_utils import run_bass_kernel_spmd
from concourse.library_config import mlp

N_NODES = 100000
N_EDGES = 1600000
IN_F = 128
H = 8
F = 16
HF = H * F  # 128
NEG_SLOPE = 0.2
EPS = 1e-16
N_CORES = 8
TGT_PER_CORE = N_NODES // N_CORES  # 12500
N_BLOCKS = cdiv(TGT_PER_CORE, 128)  # 98
TGT_PAD = N_BLOCKS * 128  # 12544
CHUNK = 196 * 128  # 25088 nodes per chunk (tile aligned)
N_CHUNKS = 4
CHUNK_NODES = [CHUNK, CHUNK, CHUNK, N_NODES - 3 * CHUNK]  # last: 24736
CHUNK_ROWS = [n + 1 for n in CHUNK_NODES]  # +1 sentinel row per chunk
CHUNK_BASE = [0]
for _c in range(1, N_CHUNKS):
    CHUNK_BASE.append(CHUNK_BASE[-1] + CHUNK_ROWS[_c - 1])
TABLE_ROWS = CHUNK_BASE[-1] + CHUNK_ROWS[-1]  # 100004
ROW_ELEMS = 256  # bf16 elems per table row (512B)
ALPHA_SENT = -300.0
N_TILES = N_NODES // 128  # 781.25 -> handled per chunk: 196,196,196,193.25?
TILES_PER_CHUNK = [n // 128 for n in CHUNK_NODES]  # [196,196,196,193]
LAST_PARTIAL = CHUNK_NODES[3] - TILES_PER_CHUNK[3] * 128  # 24736-24704=32

_COMPILED = {}


def _host_prep(x, edge_index, W_proj, W_skip, a_src, a_tgt):
    """Pure index/layout prep. Returns (common, per_core list)."""
    x = np.asarray(x, np.float32)
    ei = np.asarray(edge_index)
    src = ei[0].astype(np.int64)
    tgt = ei[1].astype(np.int64)

    xT = np.ascontiguousarray(x.T)  # [128, N]
    WprojT = np.ascontiguousarray(np.asarray(W_proj, np.float32).T)
    # block-diagonal score layouts: Ablk[hf, h] = a_src[h, f'] at hf=h*16+f'
    Ablk = np.zeros((HF, H), np.float32)
    Bblk = np.zeros((HF, H), np.float32)
    asr = np.asarray(a_src, np.float32).reshape(H, F)
    atg = np.asarray(a_tgt, np.float32).reshape(H, F)
    for h in range(H):
        Ablk[h * F:(h + 1) * F, h] = asr[h]
        Bblk[h * F:(h + 1) * F, h] = atg[h]
    ABblk = np.concatenate([Ablk, Bblk], axis=1)  # [128, 16]

    chunk_of = np.minimum(src // CHUNK, N_CHUNKS - 1)
    local_of = (src - chunk_of * CHUNK).astype(np.int64)

    cores = []
    for c in range(N_CORES):
        lo, hi = c * TGT_PER_CORE, (c + 1) * TGT_PER_CORE
        m = (tgt >= lo) & (tgt < hi)
        s_loc = local_of[m]
        s_ch = chunk_of[m]
        t_loc = (tgt[m] - lo).astype(np.int64)
        cnt = np.zeros((TGT_PER_CORE, N_CHUNKS), np.int32)
        np.add.at(cnt, (t_loc, s_ch), 1)
        order = np.lexsort((-cnt[:, 3], -cnt[:, 2], -cnt[:, 1], -cnt[:, 0]))
        rank = np.empty(TGT_PER_CORE, np.int64)
        rank[order] = np.arange(TGT_PER_CORE)
        cores.append(dict(order=order, rank=rank, cnt=cnt,
                          s_loc=s_loc, s_ch=s_ch, t_loc=t_loc))

    # common schedule: W[b, ch] = max over cores of per-block max count
    W = np.zeros((N_BLOCKS, N_CHUNKS), np.int32)
    for c in range(N_CORES):
        d = cores[c]
        cnt_ord = d["cnt"][d["order"]]  # [12500, 4]
        cnt_pad = np.zeros((TGT_PAD, N_CHUNKS), np.int32)
        cnt_pad[:TGT_PER_CORE] = cnt_ord
        blkmax = cnt_pad.reshape(N_BLOCKS, 128, N_CHUNKS).max(axis=1)
        W = np.maximum(W, blkmax)
    W[:, 0] = np.maximum(W[:, 0], 1)  # ensure every block has >=1 slot

    per_core = []
    for c in range(N_CORES):
        d = cores[c]
        # slot fill: for each (rank, ch) edges in arbitrary order
        rk = d["rank"][d["t_loc"]]  # rank of each edge's target
        # sort edges by (rank, ch) to place into slots
        eo = np.lexsort((d["s_ch"], rk))
        rk_s = rk[eo]
        ch_s = d["s_ch"][eo]
        sl_s = d["s_loc"][eo]
        # position of each edge within its (rank, ch) group
        key = rk_s * N_CHUNKS + ch_s
        uk = np.unique(key)
        firsts = np.searchsorted(key, uk)
        dpos = np.arange(len(key)) - firsts[np.searchsorted(uk, key)]
        # build idx arrays per (block, ch): [W, 128] (d-major, partition minor)
        idx_cols = []
        for b in range(N_BLOCKS):
            for ch in range(N_CHUNKS):
                w = int(W[b, ch])
                if w == 0:
                    continue
                arr = np.full((w, 128), CHUNK_NODES[ch], np.int64)  # sentinel
                mm = (rk_s // 128 == b) & (ch_s == ch)
                if mm.any():
                    p = (rk_s[mm] % 128).astype(np.int64)
                    dd = dpos[mm]
                    arr[dd, p] = sl_s[mm]
                flat = arr.reshape(-1)  # j = d*128 + p
                wrap = flat.reshape(-1, 16).T  # [16, 8w]
                idx_cols.append(np.tile(wrap, (8, 1)))  # [128, 8w]
        idxs = np.concatenate(idx_cols, axis=1).astype(np.int16)  # [128, C]

        perm = d["order"]
        xTp = np.zeros((IN_F, TGT_PAD), np.float32)
        xTp[:, :TGT_PER_CORE] = x[perm + c * TGT_PER_CORE].T
        per_core.append(dict(idxs=idxs, xTperm=np.ascontiguousarray(xTp),
                             perm=perm))

    common = dict(xT=xT, WprojT=WprojT, ABblk=ABblk,
                  Wproj=np.asarray(W_proj, np.float32),
                  Wskip=np.asarray(W_skip, np.float32), W=W)
    return common, per_core


def _build_program(W):
    nc = bacc.Bacc("TRN2", debug=False, num_devices=N_CORES,
                   num_swdge_queues=4)
    f32 = mybir.dt.float32
    bf16 = mybir.dt.bfloat16
    i16 = mybir.dt.int16

    C_total = int(8 * W.sum())
    xT = nc.dram_tensor("xT", [IN_F, N_NODES], f32, kind="ExternalInput").ap()
    xTperm = nc.dram_tensor("xTperm", [IN_F, TGT_PAD], f32,
                            kind="ExternalInput").ap()
    WprojT_d = nc.dram_tensor("WprojT", [HF, IN_F], f32,
                              kind="ExternalInput").ap()
    ABblk_d = nc.dram_tensor("ABblk", [HF, 2 * H], f32,
                             kind="ExternalInput").ap()
    Wproj_d = nc.dram_tensor("Wproj", [IN_F, HF], f32,
                             kind="ExternalInput").ap()
    Wskip_d = nc.dram_tensor("Wskip", [IN_F, HF], f32,
                             kind="ExternalInput").ap()
    idxs_d = nc.dram_tensor("idxs", [128, C_total], i16,
                            kind="ExternalInput").ap()
    out_d = nc.dram_tensor("out", [TGT_PAD, HF], f32,
                           kind="ExternalOutput").ap()
    table = nc.dram_tensor("table", [TABLE_ROWS, ROW_ELEMS], bf16).ap()

    with tile.TileContext(nc) as tc, ExitStack() as ctx:
        consts = ctx.enter_context(tc.tile_pool(name="consts", bufs=1))
        sb = ctx.enter_context(tc.tile_pool(name="sb", bufs=3))
        stg = ctx.enter_context(tc.tile_pool(name="stg", bufs=4))
        gpool = ctx.enter_context(tc.tile_pool(name="gpool", bufs=3))
        work = ctx.enter_context(tc.tile_pool(name="work", bufs=2))
        psum = ctx.enter_context(tc.tile_pool(name="psum", bufs=2,
                                              space="PSUM"))
        psum2 = ctx.enter_context(tc.tile_pool(name="psum2", bufs=2,
                                               space="PSUM"))
        idxp = ctx.enter_context(tc.tile_pool(name="idxp", bufs=2))

        nc.gpsimd.load_library(mlp)

        # --- Phase A: weights and packs -------------------------------
        wprojT_t = consts.tile([HF, IN_F], f32)
        nc.sync.dma_start(out=wprojT_t[:], in_=WprojT_d[:])
        abblk_t = consts.tile([HF, 2 * H], f32)
        nc.sync.dma_start(out=abblk_t[:], in_=ABblk_d[:])
        pack0 = consts.tile([IN_F, HF + H], f32)  # [W_proj | Wa]
        pack2 = consts.tile([IN_F, HF + H], f32)  # [W_skip | Wb]
        nc.sync.dma_start(out=pack0[:, :HF], in_=Wproj_d[:])
        nc.sync.dma_start(out=pack2[:, :HF], in_=Wskip_d[:])
        wab_ps = psum.tile([IN_F, 2 * H], f32, space="PSUM")
        nc.tensor.matmul(out=wab_ps[:], lhsT=wprojT_t[:],
                         rhs=abblk_t[:], start=True, stop=True)
        nc.vector.tensor_copy(out=pack0[:, HF:HF + H], in_=wab_ps[:, 0:H])
        nc.vector.tensor_copy(out=pack2[:, HF:HF + H], in_=wab_ps[:, H:2 * H])

        # --- Phase B: build table ------------------------------------
        for ch in range(N_CHUNKS):
            ntile = cdiv(CHUNK_NODES[ch], 128)
            for t in range(ntile):
                nrows = min(128, CHUNK_NODES[ch] - t * 128)
                n0 = ch * CHUNK + t * 128
                xt = stg.tile([IN_F, 128], f32, tag="xt")
                nc.sync.dma_start(out=xt[:, :nrows],
                                  in_=xT[:, n0:n0 + nrows])
                pa = psum.tile([128, HF + H], f32, space="PSUM", tag="pa")
                nc.tensor.matmul(out=pa[:nrows], lhsT=xt[:, :nrows],
                                 rhs=pack0[:], start=True, stop=True)
                row = stg.tile([128, ROW_ELEMS], bf16, tag="row")
                nc.scalar.activation(out=row[:nrows, 0:HF],
                                     in_=pa[:nrows, 0:HF],
                                     func=mybir.ActivationFunctionType.Copy)
                nc.vector.tensor_copy(
                    out=row[:nrows, HF:HF + 2 * H].bitcast(f32),
                    in_=pa[:nrows, HF:HF + H])
                r0 = CHUNK_BASE[ch] + t * 128
                nc.sync.dma_start(out=table[r0:r0 + nrows, :],
                                  in_=row[:nrows, :])
            # sentinel row for this chunk
            srow = stg.tile([1, ROW_ELEMS], bf16, tag="sent")
            nc.gpsimd.memset(srow[:, 0:HF], 0.0)
            nc.gpsimd.memset(srow[:, HF:HF + 2 * H].bitcast(f32), ALPHA_SENT)
            nc.gpsimd.memset(srow[:, HF + 2 * H:].bitcast(f32), 0.0)
            sr = CHUNK_BASE[ch] + CHUNK_NODES[ch]
            nc.sync.dma_start(out=table[sr:sr + 1, :], in_=srow[:])

        # --- Phase C: per-block edge processing ----------------------
        Wsum = W.sum(axis=1)
        col_off = np.concatenate(([0], np.cumsum(8 * W.reshape(-1))))
        call_i = 0
        IDX_GRP = 8  # blocks per idx load group
        for b in range(N_BLOCKS):
            if b % IDX_GRP == 0:
                g0 = int(col_off[b * N_CHUNKS])
                g1 = int(col_off[min(b + IDX_GRP, N_BLOCKS) * N_CHUNKS])
                idx_t = idxp.tile([128, g1 - g0], i16, tag="idxg")
                nc.sync.dma_start(out=idx_t[:], in_=idxs_d[:, g0:g1])
                grp_base = g0
            sw = int(Wsum[b])
            # skip | beta matmul
            sk_ps = psum2.tile([128, HF + H], f32, space="PSUM", tag="sk")
            xp = stg.tile([IN_F, 128], f32, tag="xp")
            nc.sync.dma_start(out=xp[:], in_=xTperm[:, b * 128:(b + 1) * 128])
            nc.tensor.matmul(out=sk_ps[:], lhsT=xp[:], rhs=pack2[:],
                             start=True, stop=True)
            beta = work.tile([128, H], f32, tag="beta")
            nc.vector.tensor_copy(out=beta[:], in_=sk_ps[:, HF:HF + H])

            # gathers into G [128, sw, 256]
            G = gpool.tile([128, sw, ROW_ELEMS], bf16, tag="G")
            doff = 0
            for ch in range(N_CHUNKS):
                w = int(W[b, ch])
                if w == 0:
                    continue
                ni = 128 * w
                c0 = int(col_off[b * N_CHUNKS + ch]) - grp_base
                tab_ch = table[CHUNK_BASE[ch]:CHUNK_BASE[ch] + CHUNK_ROWS[ch], :]
                nc.gpsimd.dma_gather(
                    G[:, doff:doff + w, :],
                    tab_ch,
                    idx_t[:, c0:c0 + 8 * w],
                    ni, ni, ROW_ELEMS,
                    single_packet=False,
                    queue_num=call_i % 4,
                )
                call_i += 1
                doff += w
            # alpha view: [128, sw, 8] f32
            al = G[:, :, HF:HF + 2 * H].bitcast(f32)
            s_t = work.tile([128, sw, H], f32, tag="s")
            nc.vector.tensor_tensor(
                out=s_t[:], in0=al,
                in1=beta[:].unsqueeze(1).to_broadcast([128, sw, H]),
                op=mybir.AluOpType.add)
            # exp(lrelu(z)) = exp(0.2 z) * exp(0.8 relu(z)); ACT Lrelu has a
            # fixed 0.01 slope so build it from Relu/Exp(scale=...) instead.
            e1 = work.tile([128, sw, H], f32, tag="e1")
            nc.scalar.activation(out=e1[:], in_=s_t[:],
                                 func=mybir.ActivationFunctionType.Exp,
                                 scale=NEG_SLOPE)
            r_t = work.tile([128, sw, H], f32, tag="rt")
            nc.scalar.activation(out=r_t[:], in_=s_t[:],
                                 func=mybir.ActivationFunctionType.Relu)
            e2 = work.tile([128, sw, H], f32, tag="e2")
            nc.scalar.activation(out=e2[:], in_=r_t[:],
                                 func=mybir.ActivationFunctionType.Exp,
                                 scale=1.0 - NEG_SLOPE)
            Ebf = work.tile([128, sw, H], bf16, tag="Ebf")
            nc.vector.tensor_tensor(out=Ebf[:], in0=e1[:], in1=e2[:],
                                    op=mybir.AluOpType.mult)
            # expand E to [128, sw, 128]
            Ex = work.tile([128, sw, H, F], bf16, tag="Ex")
            nc.vector.tensor_copy(
                out=Ex[:],
                in_=Ebf[:].unsqueeze(3).to_broadcast([128, sw, H, F]))
            # M = p * Ex
            M = work.tile([128, sw, HF], bf16, tag="M")
            nc.vector.tensor_tensor(out=M[:], in0=G[:, :, 0:HF], in1=Ex[:].rearrange("p w h f -> p w (h f)"),
                                    op=mybir.AluOpType.mult)
            # U[v, f] = sum_d M ; D[v, h] = sum_d E
            U = work.tile([128, HF], f32, tag="U")
            nc.vector.tensor_reduce(out=U[:], in_=M[:].transpose([0, 2, 1]),
                                    axis=mybir.AxisListType.X,
                                    op=mybir.AluOpType.add)
            D = work.tile([128, H], f32, tag="D")
            nc.vector.tensor_reduce(out=D[:], in_=Ebf[:].transpose([0, 2, 1]),
                                    axis=mybir.AxisListType.X,
                                    op=mybir.AluOpType.add)
            Dinv = work.tile([128, H], f32, tag="Dinv")
            nc.vector.tensor_scalar_add(Dinv[:], D[:], EPS)
            nc.vector.reciprocal(out=Dinv[:], in_=Dinv[:])
            O = work.tile([128, HF], f32, tag="O")
            nc.vector.tensor_tensor(
                out=O[:].rearrange("p (h f) -> p h f", h=H),
                in0=U[:].rearrange("p (h f) -> p h f", h=H),
                in1=Dinv[:].unsqueeze(2).to_broadcast([128, H, F]),
                op=mybir.AluOpType.mult)
            nc.vector.tensor_tensor(out=O[:], in0=O[:], in1=sk_ps[:, 0:HF],
                                    op=mybir.AluOpType.add)
            # ELU = (max(O,0) - 1) + exp(min(O,0))
            A_t = work.tile([128, HF], f32, tag="At")
            nc.vector.tensor_scalar(out=A_t[:], in0=O[:], scalar1=0.0,
                                    scalar2=-1.0, op0=mybir.AluOpType.max,
                                    op1=mybir.AluOpType.add)
            T_t = work.tile([128, HF], f32, tag="Tt")
            nc.vector.tensor_scalar_min(T_t[:], O[:], 0.0)
            E2 = work.tile([128, HF], f32, tag="E2")
            nc.scalar.activation(out=E2[:], in_=T_t[:],
                                 func=mybir.ActivationFunctionType.Exp)
            OUT = work.tile([128, HF], f32, tag="OUT")
            nc.vector.tensor_tensor(out=OUT[:], in0=A_t[:], in1=E2[:],
                                    op=mybir.AluOpType.add)
            nc.sync.dma_start(out=out_d[b * 128:(b + 1) * 128, :],
                              in_=OUT[:])

    nc.compile()
    return nc


def kernel(x, edge_index, W_proj, W_skip, a_src, a_tgt):
    common, per_core = _host_prep(x, edge_index, W_proj, W_skip, a_src, a_tgt)
    key = "prog"
    if key not in _COMPILED:
        _COMPILED[key] = _build_program(common["W"])
    nc = _COMPILED[key]

    in_maps = []
    for c in range(N_CORES):
        pc = per_core[c]
        in_maps.append({
            "xT": common["xT"],
            "xTperm": pc["xTperm"],
            "WprojT": common["WprojT"],
            "ABblk": common["ABblk"],
            "Wproj": common["Wproj"],
            "Wskip": common["Wskip"],
            "idxs": pc["idxs"],
        })
    trace = bool(int(os.environ.get("GAT_TRACE", "0")))
    res = run_bass_kernel_spmd(nc, in_maps, list(range(N_CORES)),
                               trace=trace)
    if trace:
        kernel.last_exec_time_ns = res.exec_time_ns
        kernel.last_mean_exec_time_ns = res.mean_exec_time_ns

    out = np.empty((N_NODES, HF), np.float32)
    for c in range(N_CORES):
        o = res.results[c]["out"]  # [12544, 128] in rank order
        perm = per_core[c]["perm"]
        out[c * TGT_PER_CORE + perm] = o[:TGT_PER_CORE]
    return out


kernel.last_exec_time_ns = None
kernel.last_mean_exec_time_ns = None



# revision 12
# speedup vs baseline: 1.1661x; 1.1661x over previous
"""GAT layer on 8 Trainium2 NeuronCores (Bass/Tile).

Strategy (target-per-partition layout, edge gathers via ANT dma_gather):
  - Targets sharded across 8 cores (12500 each). Per core, targets are
    reordered (lex4 clustering by per-chunk source counts) into 98 blocks
    of 128; block b target v lives on SBUF partition v.
  - Phase B builds a DRAM table row per node: [p bf16(128) | alpha_src
    f32(8) | junk] = 512B rows, via PE matmuls with bf16 host-transposed
    x strips (stationary = xT slice, moving = [W_proj | W_proj@Ablk]).
    Table rows are interleaved (row = strip*2048 + p*16 + k) so each
    strip write is one 8KB-contiguous descriptor per partition.
  - Chunks of 32768 rows so int16 gather indices cover them; per chunk
    the last row is stolen for the sentinel (alpha=-80 => exp(s)~0) and
    the stolen node's data is duplicated into a chunk-3 pad row.
  - Per block, per chunk, dma_gather pulls each edge slot's row into
    [128 targets, W, 256] bf16 tiles. Per-core unused tail slots carry
    idx=-1 (skipped by the DGE); their alpha region is pre-memset to
    -80 so E ~= 0.
  - s = alpha + beta[target] (broadcast per partition), exp(lrelu) via
    exp(0.2 z)*exp(0.8 relu(z)). Factored softmax: U = sum_d E*p,
    D = sum_d E, out = U/D + skip, then ELU. beta/skip come from a
    per-block matmul with stationary = SBUF-resident xTperm slice.
"""

import os
import sys

sys.path.insert(0, "/opt/trn_rl_repo")

import numpy as np
from contextlib import ExitStack

import concourse.bass as bass
import concourse.bacc as bacc
import concourse.tile as tile
from concourse import mybir
from concourse._compat import cdiv
from concourse.bass_utils import run_bass_kernel_spmd
from concourse.library_config import mlp

N_NODES = 100000
N_EDGES = 1600000
IN_F = 128
H = 8
F = 16
HF = H * F  # 128
NEG_SLOPE = 0.2
EPS = 1e-16
N_CORES = 8
TGT_PER_CORE = N_NODES // N_CORES  # 12500
N_BLOCKS = cdiv(TGT_PER_CORE, 128)  # 98
TGT_PAD = N_BLOCKS * 128  # 12544

STRIP = 2048
N_STRIPS = 49
NPAD = STRIP * N_STRIPS  # 100352
N_CHUNKS = 4
CH_ROWBASE = [0, 32768, 65536, 98304]
CH_ROWS = [32768, 32768, 32768, 2048]
ROW_ELEMS = 256  # bf16 elems per table row (512B)
ALPHA_SENT = -80.0
# stolen nodes (their natural row becomes the chunk sentinel); data
# duplicated into chunk-3 pad rows.
STOLEN = [32767, 65535, 98303]  # = 32768*(ch+1)-1
DUP_PAD_NODES = [100348, 100349, 100350]  # pad nodes whose rows host the dups
SENT_ROWS = [32767 + 32768 * ch for ch in range(3)] + [100351]
GCNT_PAD = 400  # 98*4 gather calls padded to a multiple of 16

_COMPILED = {}


def _row_of(n):
    s = n // STRIP
    w = n % STRIP
    return s * STRIP + (w % 128) * 16 + (w // 128)


def _host_prep(x, edge_index, W_proj, W_skip, a_src, a_tgt):
    """Pure index/layout prep. Returns (common, per_core list)."""
    x = np.asarray(x, np.float32)
    ei = np.asarray(edge_index)
    src = ei[0].astype(np.int64)
    tgt = ei[1].astype(np.int64)

    xT = np.zeros((IN_F, NPAD), np.float32)
    xT[:, :N_NODES] = x.T
    xT16 = _to_bf16(xT)

    # weight folding on host: pack0 = [W_proj | W_proj @ Ablk],
    # pack2 = [W_skip | W_proj @ Bblk]
    Wp = np.asarray(W_proj, np.float64)
    Ws = np.asarray(W_skip, np.float64)
    asr = np.asarray(a_src, np.float64).reshape(H, F)
    atg = np.asarray(a_tgt, np.float64).reshape(H, F)
    Ablk = np.zeros((HF, H))
    Bblk = np.zeros((HF, H))
    for h in range(H):
        Ablk[h * F:(h + 1) * F, h] = asr[h]
        Bblk[h * F:(h + 1) * F, h] = atg[h]
    pack0 = _to_bf16(np.concatenate([Wp, Wp @ Ablk], axis=1).astype(np.float32))
    pack2 = _to_bf16(np.concatenate([Ws, Wp @ Bblk], axis=1).astype(np.float32))

    # per-edge chunk + chunk-relative idx (with stolen-node redirect)
    chunk_of = np.minimum(src // 32768, 3)
    rows = _row_of(src)
    idxval = rows - np.take(np.array(CH_ROWBASE, np.int64), chunk_of)
    for i, xn in enumerate(STOLEN):
        m = src == xn
        chunk_of[m] = 3
        idxval[m] = _row_of(DUP_PAD_NODES[i]) - CH_ROWBASE[3]
    sent_idx = np.array([32767, 32767, 32767, 100351 - CH_ROWBASE[3]], np.int64)

    cores = []
    for c in range(N_CORES):
        lo, hi = c * TGT_PER_CORE, (c + 1) * TGT_PER_CORE
        m = (tgt >= lo) & (tgt < hi)
        s_idx = idxval[m]
        s_ch = chunk_of[m]
        t_loc = (tgt[m] - lo).astype(np.int64)
        cnt = np.zeros((TGT_PER_CORE, N_CHUNKS), np.int32)
        np.add.at(cnt, (t_loc, s_ch), 1)
        order = np.lexsort((-cnt[:, 3], -cnt[:, 2], -cnt[:, 1], -cnt[:, 0]))
        rank = np.empty(TGT_PER_CORE, np.int64)
        rank[order] = np.arange(TGT_PER_CORE)
        cnt_pad = np.zeros((TGT_PAD, N_CHUNKS), np.int32)
        cnt_pad[:TGT_PER_CORE] = cnt[order]
        wc = np.maximum(cnt_pad.reshape(N_BLOCKS, 128, N_CHUNKS).max(axis=1), 1)
        cores.append(dict(order=order, rank=rank, wc=wc,
                          s_idx=s_idx, s_ch=s_ch, t_loc=t_loc))

    W = np.zeros((N_BLOCKS, N_CHUNKS), np.int32)
    for c in range(N_CORES):
        W = np.maximum(W, cores[c]["wc"])

    per_core = []
    for c in range(N_CORES):
        d = cores[c]
        rk = d["rank"][d["t_loc"]]
        eo = np.lexsort((d["s_ch"], rk))
        rk_s = rk[eo]
        ch_s = d["s_ch"][eo]
        sl_s = d["s_idx"][eo]
        key = rk_s * N_CHUNKS + ch_s
        uk = np.unique(key)
        firsts = np.searchsorted(key, uk)
        dpos = np.arange(len(key)) - firsts[np.searchsorted(uk, key)]
        wc = d["wc"]
        idx_cols = []
        for b in range(N_BLOCKS):
            for ch in range(N_CHUNKS):
                w = int(W[b, ch])
                arr = np.full((w, 128), sent_idx[ch], np.int64)
                arr[int(wc[b, ch]):, :] = -1  # per-core unused tail
                mm = (rk_s // 128 == b) & (ch_s == ch)
                if mm.any():
                    p = (rk_s[mm] % 128).astype(np.int64)
                    dd = dpos[mm]
                    arr[dd, p] = sl_s[mm]
                flat = arr.reshape(-1)  # j = d*128 + p
                wrap = flat.reshape(-1, 16).T  # [16, 8w]
                idx_cols.append(np.tile(wrap, (8, 1)))  # [128, 8w]
        idxs = np.concatenate(idx_cols, axis=1).astype(np.int16)
        gcnt = np.zeros(GCNT_PAD, np.int32)
        gcnt[:N_BLOCKS * N_CHUNKS] = (128 * wc).reshape(-1)

        perm = d["order"]
        xTp = np.zeros((IN_F, TGT_PAD), np.float32)
        xTp[:, :TGT_PER_CORE] = x[perm + c * TGT_PER_CORE].T
        per_core.append(dict(idxs=idxs, xTperm=_to_bf16(xTp), perm=perm,
                             gcnt=gcnt.reshape(1, -1)))

    common = dict(xT=xT16, pack0=pack0, pack2=pack2, W=W)
    return common, per_core


def _to_bf16(a):
    import ml_dtypes
    return np.ascontiguousarray(np.asarray(a, np.float32)).astype(
        ml_dtypes.bfloat16)


def _build_program(W):
    nc = bacc.Bacc("TRN2", debug=False, num_devices=N_CORES,
                   num_swdge_queues=4)
    f32 = mybir.dt.float32
    bf16 = mybir.dt.bfloat16
    i16 = mybir.dt.int16

    C_total = int(8 * W.sum())
    SWMAX = int(W.sum(axis=1).max())
    Wsum = W.sum(axis=1)

    xT_d = nc.dram_tensor("xT", [IN_F, NPAD], bf16, kind="ExternalInput").ap()
    xTperm_d = nc.dram_tensor("xTperm", [IN_F, TGT_PAD], bf16,
                              kind="ExternalInput").ap()
    pack0_d = nc.dram_tensor("pack0", [IN_F, HF + H], bf16,
                             kind="ExternalInput").ap()
    pack2_d = nc.dram_tensor("pack2", [IN_F, HF + H], bf16,
                             kind="ExternalInput").ap()
    idxs_d = nc.dram_tensor("idxs", [128, C_total], i16,
                            kind="ExternalInput").ap()
    gcnt_d = nc.dram_tensor("gcnt", [1, GCNT_PAD], mybir.dt.int32,
                            kind="ExternalInput").ap()
    out_d = nc.dram_tensor("out", [TGT_PAD, HF], f32,
                           kind="ExternalOutput").ap()
    table = nc.dram_tensor("table", [NPAD, ROW_ELEMS], bf16).ap()

    with tile.TileContext(nc) as tc, ExitStack() as ctx:
        consts = ctx.enter_context(tc.tile_pool(name="consts", bufs=1))
        stg = ctx.enter_context(tc.tile_pool(name="stg", bufs=2))
        rowp = ctx.enter_context(tc.tile_pool(name="rowp", bufs=2))
        gpool = ctx.enter_context(tc.tile_pool(name="gpool", bufs=2))
        work = ctx.enter_context(tc.tile_pool(name="work", bufs=2))
        psA = ctx.enter_context(tc.tile_pool(name="psA", bufs=2, space="PSUM"))
        psC = ctx.enter_context(tc.tile_pool(name="psC", bufs=2, space="PSUM"))
        idxp = ctx.enter_context(tc.tile_pool(name="idxp", bufs=2))

        nc.gpsimd.load_library(mlp)

        # --- constants ------------------------------------------------
        pack0_t = consts.tile([IN_F, HF + H], bf16)
        nc.sync.dma_start(out=pack0_t[:], in_=pack0_d[:])
        pack2_t = consts.tile([IN_F, HF + H], bf16)
        nc.sync.dma_start(out=pack2_t[:], in_=pack2_d[:])
        xTperm_t = consts.tile([IN_F, TGT_PAD], bf16)
        nc.sync.dma_start(out=xTperm_t[:], in_=xTperm_d[:])
        gcnt_t = consts.tile([1, GCNT_PAD], mybir.dt.int32)
        nc.sync.dma_start(out=gcnt_t[:], in_=gcnt_d[:])
        cregs = [nc.gpsimd.alloc_register(f"gc{i}") for i in range(16)]
        sent_t = consts.tile([1, ROW_ELEMS], bf16)
        nc.gpsimd.memset(sent_t[:, 0:HF], 0.0)
        nc.gpsimd.memset(sent_t[:, HF:HF + 2 * H].bitcast(f32), ALPHA_SENT)
        nc.gpsimd.memset(sent_t[:, HF + 2 * H:].bitcast(f32), 0.0)

        # --- Phase B: build table ------------------------------------
        for s in range(N_STRIPS):
            xs = stg.tile([IN_F, STRIP], bf16, tag="xs")
            nc.scalar.dma_start(out=xs[:],
                                in_=xT_d[:, s * STRIP:(s + 1) * STRIP])
            rb = rowp.tile([128, 16, ROW_ELEMS], bf16, tag="rb")
            if s < 2:
                nc.vector.memset(rb[:, :, HF + 2 * H:], 0.0)
            for k in range(16):
                pa = psA.tile([128, HF + H], f32, space="PSUM", tag="pa")
                nc.tensor.matmul(out=pa[:], lhsT=xs[:, k * 128:(k + 1) * 128],
                                 rhs=pack0_t[:], start=True, stop=True)
                nc.scalar.activation(out=rb[:, k, 0:HF], in_=pa[:, 0:HF],
                                     func=mybir.ActivationFunctionType.Copy)
                nc.vector.tensor_copy(
                    out=rb[:, k, HF:HF + 2 * H].bitcast(f32),
                    in_=pa[:, HF:HF + H])
            # rows s*2048 + p*16 + k <- rb[p, k]; contiguous 8KB/partition
            nc.sync.dma_start(
                out=table[s * STRIP:(s + 1) * STRIP, :].rearrange(
                    "(p k) e -> p k e", k=16),
                in_=rb[:])
        # duplicate stolen-node rows into chunk-3 pad rows (DRAM->DRAM)
        for i in range(3):
            sr = _row_of(STOLEN[i])
            dr = _row_of(DUP_PAD_NODES[i])
            nc.sync.dma_start(out=table[dr:dr + 1, :], in_=table[sr:sr + 1, :])
        # sentinel rows (overwrite stolen rows + one pad row)
        for r in SENT_ROWS:
            nc.sync.dma_start(out=table[r:r + 1, :], in_=sent_t[:])

        # --- Phase C: per-block edge processing ----------------------
        col_off = np.concatenate(([0], np.cumsum(8 * W.reshape(-1))))
        call_i = 0
        IDX_GRP = 8
        for b in range(N_BLOCKS):
            if b % IDX_GRP == 0:
                g0 = int(col_off[b * N_CHUNKS])
                g1 = int(col_off[min(b + IDX_GRP, N_BLOCKS) * N_CHUNKS])
                idx_t = idxp.tile([128, g1 - g0], i16, tag="idxg")
                nc.sync.dma_start(out=idx_t[:], in_=idxs_d[:, g0:g1])
                grp_base = g0
            sw = int(Wsum[b])
            # skip | beta matmul (stationary straight from SBUF consts)
            sk_ps = psC.tile([128, HF + H], f32, space="PSUM", tag="sk")
            nc.tensor.matmul(out=sk_ps[:],
                             lhsT=xTperm_t[:, b * 128:(b + 1) * 128],
                             rhs=pack2_t[:], start=True, stop=True)
            beta = work.tile([128, H], f32, tag="beta")
            nc.vector.tensor_copy(out=beta[:], in_=sk_ps[:, HF:HF + H])

            G = gpool.tile([128, SWMAX, ROW_ELEMS], bf16, tag="G")
            if b < 2:
                # pristine SBUF could hold inf/nan bit patterns
                nc.vector.memset(G[:, :, 0:HF], 0.0)
            # neutralize per-core tail slots (idx=-1 leaves stale data)
            nc.vector.memset(G[:, 0:sw, HF:HF + 2 * H], ALPHA_SENT)
            doff = 0
            for ch in range(N_CHUNKS):
                w = int(W[b, ch])
                ni = 128 * w
                if call_i % 16 == 0:
                    nc.gpsimd.reg_load(
                        cregs, gcnt_t[0:1, call_i:call_i + 16])
                c0 = int(col_off[b * N_CHUNKS + ch]) - grp_base
                tab_ch = table[CH_ROWBASE[ch]:CH_ROWBASE[ch] + CH_ROWS[ch], :]
                nc.gpsimd.dma_gather(
                    G[:, doff:doff + w, :],
                    tab_ch,
                    idx_t[:, c0:c0 + 8 * w],
                    ni, cregs[call_i % 16], ROW_ELEMS,
                    single_packet=False,
                    queue_num=call_i % 4,
                )
                call_i += 1
                doff += w
            al = G[:, 0:sw, HF:HF + 2 * H].bitcast(f32)  # [128, sw, 8]
            s_t = work.tile([128, sw, H], f32, tag="s")
            nc.vector.tensor_tensor(
                out=s_t[:], in0=al,
                in1=beta[:].unsqueeze(1).to_broadcast([128, sw, H]),
                op=mybir.AluOpType.add)
            # exp(lrelu(z)) = exp(0.2 z) * exp(0.8 relu(z))
            e1 = work.tile([128, sw, H], f32, tag="e1")
            nc.scalar.activation(out=e1[:], in_=s_t[:],
                                 func=mybir.ActivationFunctionType.Exp,
                                 scale=NEG_SLOPE)
            r_t = work.tile([128, sw, H], f32, tag="rt")
            nc.scalar.activation(out=r_t[:], in_=s_t[:],
                                 func=mybir.ActivationFunctionType.Relu)
            e2 = work.tile([128, sw, H], f32, tag="e2")
            nc.scalar.activation(out=e2[:], in_=r_t[:],
                                 func=mybir.ActivationFunctionType.Exp,
                                 scale=1.0 - NEG_SLOPE)
            Ebf = work.tile([128, sw, H], bf16, tag="Ebf")
            nc.vector.tensor_tensor(out=Ebf[:], in0=e1[:], in1=e2[:],
                                    op=mybir.AluOpType.mult)
            # M = p * E (broadcast E over f)
            M = work.tile([128, sw, H, F], bf16, tag="M")
            nc.vector.tensor_tensor(
                out=M[:],
                in0=G[:, 0:sw, 0:HF].rearrange("p w (h f) -> p w h f", h=H),
                in1=Ebf[:].unsqueeze(3).to_broadcast([128, sw, H, F]),
                op=mybir.AluOpType.mult)
            # U[v, hf] = sum_d M ; D[v, h] = sum_d E
            U = work.tile([128, HF], f32, tag="U")
            nc.vector.tensor_reduce(
                out=U[:],
                in_=M[:].rearrange("p w h f -> p w (h f)").transpose([0, 2, 1]),
                axis=mybir.AxisListType.X, op=mybir.AluOpType.add)
            D = work.tile([128, H], f32, tag="D")
            nc.vector.tensor_reduce(out=D[:], in_=Ebf[:].transpose([0, 2, 1]),
                                    axis=mybir.AxisListType.X,
                                    op=mybir.AluOpType.add)
            Dinv = work.tile([128, H], f32, tag="Dinv")
            nc.vector.tensor_scalar_add(Dinv[:], D[:], EPS)
            nc.vector.reciprocal(out=Dinv[:], in_=Dinv[:])
            O = work.tile([128, HF], f32, tag="O")
            nc.vector.tensor_tensor(
                out=O[:].rearrange("p (h f) -> p h f", h=H),
                in0=U[:].rearrange("p (h f) -> p h f", h=H),
                in1=Dinv[:].unsqueeze(2).to_broadcast([128, H, F]),
                op=mybir.AluOpType.mult)
            nc.vector.tensor_tensor(out=O[:], in0=O[:], in1=sk_ps[:, 0:HF],
                                    op=mybir.AluOpType.add)
            # ELU = (max(O,0) - 1) + exp(min(O,0))
            A_t = work.tile([128, HF], f32, tag="At")
            nc.vector.tensor_scalar(out=A_t[:], in0=O[:], scalar1=0.0,
                                    scalar2=-1.0, op0=mybir.AluOpType.max,
                                    op1=mybir.AluOpType.add)
            T_t = work.tile([128, HF], f32, tag="Tt")
            nc.vector.tensor_scalar_min(T_t[:], O[:], 0.0)
            E2 = work.tile([128, HF], f32, tag="E2")
            nc.scalar.activation(out=E2[:], in_=T_t[:],
                                 func=mybir.ActivationFunctionType.Exp)
            OUT = work.tile([128, HF], f32, tag="OUT")
            nc.vector.tensor_tensor(out=OUT[:], in0=A_t[:], in1=E2[:],
                                    op=mybir.AluOpType.add)
            nc.sync.dma_start(out=out_d[b * 128:(b + 1) * 128, :],
                              in_=OUT[:])

    nc.compile()
    return nc


def kernel(x, edge_index, W_proj, W_skip, a_src, a_tgt):
    common, per_core = _host_prep(x, edge_index, W_proj, W_skip, a_src, a_tgt)
    key = "prog"
    if key not in _COMPILED:
        _COMPILED[key] = _build_program(common["W"])
    nc = _COMPILED[key]

    in_maps = []
    for c in range(N_CORES):
        pc = per_core[c]
        in_maps.append({
            "xT": common["xT"],
            "xTperm": pc["xTperm"],
            "pack0": common["pack0"],
            "pack2": common["pack2"],
            "idxs": pc["idxs"],
            "gcnt": pc["gcnt"],
        })
    trace = bool(int(os.environ.get("GAT_TRACE", "0")))
    res = run_bass_kernel_spmd(nc, in_maps, list(range(N_CORES)),
                               trace=trace)
    if trace:
        kernel.last_exec_time_ns = res.exec_time_ns
        kernel.last_mean_exec_time_ns = res.mean_exec_time_ns

    out = np.empty((N_NODES, HF), np.float32)
    for c in range(N_CORES):
        o = res.results[c]["out"]  # [12544, 128] in rank order
        perm = per_core[c]["perm"]
        out[c * TGT_PER_CORE + perm] = o[:TGT_PER_CORE]
    return out


kernel.last_exec_time_ns = None
kernel.last_mean_exec_time_ns = None


# revision 19
# speedup vs baseline: 1.3027x; 1.1172x over previous
"""GAT layer on 8 Trainium2 NeuronCores (Bass/Tile).

Strategy (target-per-partition layout, edge gathers via ANT dma_gather):
  - Targets sharded across 8 cores (12500 each). Per core, targets are
    reordered (lex4 clustering by per-chunk source counts) into 98 blocks
    of 128; block b target v lives on SBUF partition v.
  - Phase B builds a DRAM table row per node: [p bf16(128) | alpha_src
    f32(8) | junk] = 512B rows, via PE matmuls with bf16 host-transposed
    x strips (stationary = xT slice, moving = [W_proj | W_proj@Ablk]).
    Table rows are interleaved (row = strip*2048 + p*16 + k) so each
    strip write is one 8KB-contiguous descriptor per partition.
  - Chunks of 32768 rows so int16 gather indices cover them; per chunk
    the last row is stolen for the sentinel (alpha=-80 => exp(s)~0) and
    the stolen node's data is duplicated into a chunk-3 pad row.
  - Per block, per chunk, dma_gather pulls each edge slot's row into
    [128 targets, W, 256] bf16 tiles. Per-core unused tail slots carry
    idx=-1 (skipped by the DGE); their alpha region is pre-memset to
    -80 so E ~= 0.
  - s = alpha + beta[target] (broadcast per partition), exp(lrelu) via
    exp(0.2 z)*exp(0.8 relu(z)). Factored softmax: U = sum_d E*p,
    D = sum_d E, out = U/D + skip, then ELU. beta/skip come from a
    per-block matmul with stationary = SBUF-resident xTperm slice.
"""

import os
import sys

sys.path.insert(0, "/opt/trn_rl_repo")

import numpy as np
from contextlib import ExitStack

import concourse.bass as bass
import concourse.bacc as bacc
import concourse.tile as tile
from concourse import mybir
from concourse._compat import cdiv
from concourse.bass_utils import run_bass_kernel_spmd
from concourse.library_config import mlp

N_NODES = 100000
N_EDGES = 1600000
IN_F = 128
H = 8
F = 16
HF = H * F  # 128
NEG_SLOPE = 0.2
EPS = 1e-16
N_CORES = 8
TGT_PER_CORE = N_NODES // N_CORES  # 12500
N_BLOCKS = cdiv(TGT_PER_CORE, 128)  # 98
TGT_PAD = N_BLOCKS * 128  # 12544

STRIP = 2048
N_STRIPS = 49
NPAD = STRIP * N_STRIPS  # 100352
N_CHUNKS = 4
CH_ROWBASE = [0, 32768, 65536, 98304]
CH_ROWS = [32768, 32768, 32768, 2048]
ROW_ELEMS = 256  # bf16 elems per table row (512B)
ALPHA_SENT = -80.0
# stolen nodes (their natural row becomes the chunk sentinel); data
# duplicated into chunk-3 pad rows.
STOLEN = [32767, 65535, 98303]  # = 32768*(ch+1)-1
DUP_PAD_NODES = [100348, 100349, 100350]  # pad nodes whose rows host the dups
SENT_ROWS = [32767 + 32768 * ch for ch in range(3)] + [100351]
GCNT_PAD = 400  # 98*4 gather calls padded to a multiple of 16

_COMPILED = {}


def _row_of(n):
    s = n // STRIP
    w = n % STRIP
    return s * STRIP + (w % 128) * 16 + (w // 128)


def _host_prep(x, edge_index, W_proj, W_skip, a_src, a_tgt):
    """Pure index/layout prep. Returns (common, per_core list)."""
    x = np.asarray(x, np.float32)
    ei = np.asarray(edge_index)
    src = ei[0].astype(np.int64)
    tgt = ei[1].astype(np.int64)

    xT = np.zeros((IN_F, NPAD), np.float32)
    xT[:, :N_NODES] = x.T
    xT16 = _to_bf16(xT)

    # weight folding on host: pack0 = [W_proj | W_proj @ Ablk],
    # pack2 = [W_skip | W_proj @ Bblk]
    Wp = np.asarray(W_proj, np.float64)
    Ws = np.asarray(W_skip, np.float64)
    asr = np.asarray(a_src, np.float64).reshape(H, F)
    atg = np.asarray(a_tgt, np.float64).reshape(H, F)
    Ablk = np.zeros((HF, H))
    Bblk = np.zeros((HF, H))
    for h in range(H):
        Ablk[h * F:(h + 1) * F, h] = asr[h]
        Bblk[h * F:(h + 1) * F, h] = atg[h]
    pack0 = _to_bf16(np.concatenate([Wp, Wp @ Ablk], axis=1).astype(np.float32))
    pack2 = _to_bf16(np.concatenate([Ws, Wp @ Bblk], axis=1).astype(np.float32))

    # per-edge chunk + chunk-relative idx (with stolen-node redirect)
    chunk_of = np.minimum(src // 32768, 3)
    rows = _row_of(src)
    idxval = rows - np.take(np.array(CH_ROWBASE, np.int64), chunk_of)
    for i, xn in enumerate(STOLEN):
        m = src == xn
        chunk_of[m] = 3
        idxval[m] = _row_of(DUP_PAD_NODES[i]) - CH_ROWBASE[3]
    sent_idx = np.array([32767, 32767, 32767, 100351 - CH_ROWBASE[3]], np.int64)

    cores = []
    for c in range(N_CORES):
        lo, hi = c * TGT_PER_CORE, (c + 1) * TGT_PER_CORE
        m = (tgt >= lo) & (tgt < hi)
        s_idx = idxval[m]
        s_ch = chunk_of[m]
        t_loc = (tgt[m] - lo).astype(np.int64)
        cnt = np.zeros((TGT_PER_CORE, N_CHUNKS), np.int32)
        np.add.at(cnt, (t_loc, s_ch), 1)
        order = np.lexsort((-cnt[:, 3], -cnt[:, 2], -cnt[:, 1], -cnt[:, 0]))
        rank = np.empty(TGT_PER_CORE, np.int64)
        rank[order] = np.arange(TGT_PER_CORE)
        cnt_pad = np.zeros((TGT_PAD, N_CHUNKS), np.int32)
        cnt_pad[:TGT_PER_CORE] = cnt[order]
        wc = np.maximum(cnt_pad.reshape(N_BLOCKS, 128, N_CHUNKS).max(axis=1), 1)
        cores.append(dict(order=order, rank=rank, wc=wc,
                          s_idx=s_idx, s_ch=s_ch, t_loc=t_loc))

    W = np.zeros((N_BLOCKS, N_CHUNKS), np.int32)
    for c in range(N_CORES):
        W = np.maximum(W, cores[c]["wc"])

    per_core = []
    for c in range(N_CORES):
        d = cores[c]
        rk = d["rank"][d["t_loc"]]
        eo = np.lexsort((d["s_ch"], rk))
        rk_s = rk[eo]
        ch_s = d["s_ch"][eo]
        sl_s = d["s_idx"][eo]
        key = rk_s * N_CHUNKS + ch_s
        uk = np.unique(key)
        firsts = np.searchsorted(key, uk)
        dpos = np.arange(len(key)) - firsts[np.searchsorted(uk, key)]
        wc = d["wc"]
        idx_cols = []
        for b in range(N_BLOCKS):
            for ch in range(N_CHUNKS):
                w = int(W[b, ch])
                arr = np.full((w, 128), sent_idx[ch], np.int64)
                arr[int(wc[b, ch]):, :] = -1  # per-core unused tail
                mm = (rk_s // 128 == b) & (ch_s == ch)
                if mm.any():
                    p = (rk_s[mm] % 128).astype(np.int64)
                    dd = dpos[mm]
                    arr[dd, p] = sl_s[mm]
                flat = arr.reshape(-1)  # j = d*128 + p
                wrap = flat.reshape(-1, 16).T  # [16, 8w]
                idx_cols.append(np.tile(wrap, (8, 1)))  # [128, 8w]
        idxs = np.concatenate(idx_cols, axis=1).astype(np.int16)
        gcnt = np.zeros(GCNT_PAD, np.int32)
        gcnt[:N_BLOCKS * N_CHUNKS] = (128 * wc).reshape(-1)

        perm = d["order"]
        xTp = np.zeros((IN_F, TGT_PAD), np.float32)
        xTp[:, :TGT_PER_CORE] = x[perm + c * TGT_PER_CORE].T
        per_core.append(dict(idxs=idxs, xTperm=_to_bf16(xTp), perm=perm,
                             gcnt=gcnt.reshape(1, -1)))

    common = dict(xT=xT16, pack0=pack0, pack2=pack2, W=W)
    return common, per_core


def _to_bf16(a):
    import ml_dtypes
    return np.ascontiguousarray(np.asarray(a, np.float32)).astype(
        ml_dtypes.bfloat16)


def _build_program(W):
    nc = bacc.Bacc("TRN2", debug=False, num_devices=N_CORES,
                   num_swdge_queues=4)
    f32 = mybir.dt.float32
    bf16 = mybir.dt.bfloat16
    i16 = mybir.dt.int16

    C_total = int(8 * W.sum())
    SWMAX = int(W.sum(axis=1).max())
    Wsum = W.sum(axis=1)

    xT_d = nc.dram_tensor("xT", [IN_F, NPAD], bf16, kind="ExternalInput").ap()
    xTperm_d = nc.dram_tensor("xTperm", [IN_F, TGT_PAD], bf16,
                              kind="ExternalInput").ap()
    pack0_d = nc.dram_tensor("pack0", [IN_F, HF + H], bf16,
                             kind="ExternalInput").ap()
    pack2_d = nc.dram_tensor("pack2", [IN_F, HF + H], bf16,
                             kind="ExternalInput").ap()
    idxs_d = nc.dram_tensor("idxs", [128, C_total], i16,
                            kind="ExternalInput").ap()
    gcnt_d = nc.dram_tensor("gcnt", [1, GCNT_PAD], mybir.dt.int32,
                            kind="ExternalInput").ap()
    out_d = nc.dram_tensor("out", [TGT_PAD, HF], f32,
                           kind="ExternalOutput").ap()
    table = nc.dram_tensor("table", [NPAD, ROW_ELEMS], bf16).ap()

    with tile.TileContext(nc) as tc, ExitStack() as ctx:
        consts = ctx.enter_context(tc.tile_pool(name="consts", bufs=1))
        stg = ctx.enter_context(tc.tile_pool(name="stg", bufs=2))
        rowp = ctx.enter_context(tc.tile_pool(name="rowp", bufs=2))
        gpool = ctx.enter_context(tc.tile_pool(name="gpool", bufs=3))
        work = ctx.enter_context(tc.tile_pool(name="work", bufs=2))
        psA = ctx.enter_context(tc.tile_pool(name="psA", bufs=2, space="PSUM"))
        psC = ctx.enter_context(tc.tile_pool(name="psC", bufs=2, space="PSUM"))
        idxp = ctx.enter_context(tc.tile_pool(name="idxp", bufs=2))

        nc.gpsimd.load_library(mlp)

        # --- constants ------------------------------------------------
        pack0_t = consts.tile([IN_F, HF + H], bf16)
        nc.sync.dma_start(out=pack0_t[:], in_=pack0_d[:])
        pack2_t = consts.tile([IN_F, HF + H], bf16)
        nc.sync.dma_start(out=pack2_t[:], in_=pack2_d[:])
        xTperm_t = consts.tile([IN_F, TGT_PAD], bf16)
        nc.sync.dma_start(out=xTperm_t[:], in_=xTperm_d[:])
        gcnt_t = consts.tile([1, GCNT_PAD], mybir.dt.int32)
        nc.sync.dma_start(out=gcnt_t[:], in_=gcnt_d[:])
        cregs = [nc.gpsimd.alloc_register(f"gc{i}") for i in range(16)]
        zc = consts.tile([128, 1], bf16)
        nc.vector.memset(zc[:], 0.0)
        sent_t = consts.tile([1, ROW_ELEMS], bf16)
        nc.gpsimd.memset(sent_t[:, 0:HF], 0.0)
        nc.gpsimd.memset(sent_t[:, HF:HF + 2 * H].bitcast(f32), ALPHA_SENT)
        nc.gpsimd.memset(sent_t[:, HF + 2 * H:].bitcast(f32), 0.0)

        # --- Phase B: build table ------------------------------------
        for s in range(N_STRIPS):
            xs = stg.tile([IN_F, STRIP], bf16, tag="xs")
            nc.scalar.dma_start(out=xs[:],
                                in_=xT_d[:, s * STRIP:(s + 1) * STRIP])
            rb = rowp.tile([128, 16, ROW_ELEMS], bf16, tag="rb")
            if s < 2:
                nc.vector.memset(rb[:, :, HF + 2 * H:], 0.0)
            for k in range(16):
                pa = psA.tile([128, HF + H], f32, space="PSUM", tag="pa")
                nc.tensor.matmul(out=pa[:], lhsT=xs[:, k * 128:(k + 1) * 128],
                                 rhs=pack0_t[:], start=True, stop=True)
                nc.scalar.activation(out=rb[:, k, 0:HF], in_=pa[:, 0:HF],
                                     func=mybir.ActivationFunctionType.Copy)
                nc.vector.tensor_copy(
                    out=rb[:, k, HF:HF + 2 * H].bitcast(f32),
                    in_=pa[:, HF:HF + H])
            # rows s*2048 + p*16 + k <- rb[p, k]; contiguous 8KB/partition
            nc.sync.dma_start(
                out=table[s * STRIP:(s + 1) * STRIP, :].rearrange(
                    "(p k) e -> p k e", k=16),
                in_=rb[:])
        # duplicate stolen-node rows into chunk-3 pad rows (DRAM->DRAM)
        for i in range(3):
            sr = _row_of(STOLEN[i])
            dr = _row_of(DUP_PAD_NODES[i])
            nc.sync.dma_start(out=table[dr:dr + 1, :], in_=table[sr:sr + 1, :])
        # sentinel rows (overwrite stolen rows + one pad row)
        for r in SENT_ROWS:
            nc.sync.dma_start(out=table[r:r + 1, :], in_=sent_t[:])

        # --- Phase C: per-block edge processing ----------------------
        col_off = np.concatenate(([0], np.cumsum(8 * W.reshape(-1))))
        call_i = 0
        IDX_GRP = 8
        for b in range(N_BLOCKS):
            if b % IDX_GRP == 0:
                g0 = int(col_off[b * N_CHUNKS])
                g1 = int(col_off[min(b + IDX_GRP, N_BLOCKS) * N_CHUNKS])
                idx_t = idxp.tile([128, g1 - g0], i16, tag="idxg")
                nc.sync.dma_start(out=idx_t[:], in_=idxs_d[:, g0:g1])
                grp_base = g0
            sw = int(Wsum[b])
            # skip | beta matmul (stationary straight from SBUF consts)
            sk_ps = psC.tile([128, HF + H], f32, space="PSUM", tag="sk")
            nc.tensor.matmul(out=sk_ps[:],
                             lhsT=xTperm_t[:, b * 128:(b + 1) * 128],
                             rhs=pack2_t[:], start=True, stop=True)
            beta = work.tile([128, H], f32, tag="beta")
            nc.scalar.activation(out=beta[:], in_=sk_ps[:, HF:HF + H],
                                 func=mybir.ActivationFunctionType.Copy)

            G = gpool.tile([128, SWMAX, ROW_ELEMS], bf16, tag="G")
            if b < 3:
                # pristine SBUF could hold inf/nan bit patterns
                nc.vector.memset(G[:, :, 0:HF], 0.0)
            # neutralize per-core tail slots (idx=-1 leaves stale data);
            # ACT "memset": Copy(0*zc + ALPHA_SENT)
            nc.scalar.activation(
                out=G[:, 0:sw, HF:HF + 2 * H],
                in_=zc[:].unsqueeze(2).to_broadcast([128, sw, 2 * H]),
                func=mybir.ActivationFunctionType.Copy, bias=ALPHA_SENT)
            doff = 0
            for ch in range(N_CHUNKS):
                w = int(W[b, ch])
                ni = 128 * w
                if call_i % 16 == 0:
                    nc.gpsimd.reg_load(
                        cregs, gcnt_t[0:1, call_i:call_i + 16])
                c0 = int(col_off[b * N_CHUNKS + ch]) - grp_base
                tab_ch = table[CH_ROWBASE[ch]:CH_ROWBASE[ch] + CH_ROWS[ch], :]
                nc.gpsimd.dma_gather(
                    G[:, doff:doff + w, :],
                    tab_ch,
                    idx_t[:, c0:c0 + 8 * w],
                    ni, cregs[call_i % 16], ROW_ELEMS,
                    single_packet=False,
                    queue_num=call_i % 4,
                )
                call_i += 1
                doff += w
            al = G[:, 0:sw, HF:HF + 2 * H].bitcast(f32)  # [128, sw, 8]
            s_t = work.tile([128, sw, H], f32, tag="s")
            nc.vector.tensor_tensor(
                out=s_t[:], in0=al,
                in1=beta[:].unsqueeze(1).to_broadcast([128, sw, H]),
                op=mybir.AluOpType.add)
            # exp(lrelu(z)) = exp(0.2 z) * exp(0.8 relu(z))
            e1 = work.tile([128, sw, H], f32, tag="e1")
            nc.scalar.activation(out=e1[:], in_=s_t[:],
                                 func=mybir.ActivationFunctionType.Exp,
                                 scale=NEG_SLOPE)
            r_t = work.tile([128, sw, H], f32, tag="rt")
            nc.scalar.activation(out=r_t[:], in_=s_t[:],
                                 func=mybir.ActivationFunctionType.Relu)
            e2 = work.tile([128, sw, H], f32, tag="e2")
            nc.scalar.activation(out=e2[:], in_=r_t[:],
                                 func=mybir.ActivationFunctionType.Exp,
                                 scale=1.0 - NEG_SLOPE)
            Ebf = work.tile([128, sw, H], bf16, tag="Ebf")
            nc.vector.tensor_tensor(out=Ebf[:], in0=e1[:], in1=e2[:],
                                    op=mybir.AluOpType.mult)
            # M = p * E (broadcast E over f)
            M = work.tile([128, sw, H, F], bf16, tag="M")
            nc.vector.tensor_tensor(
                out=M[:],
                in0=G[:, 0:sw, 0:HF].rearrange("p w (h f) -> p w h f", h=H),
                in1=Ebf[:].unsqueeze(3).to_broadcast([128, sw, H, F]),
                op=mybir.AluOpType.mult)
            # U[v, hf] = sum_d M (in-place pairwise tree in bf16, last
            # level into f32) ; D[v, h] = sum_d E
            Mflat = M[:].rearrange("p w h f -> p w (h f)")
            U = work.tile([128, HF], f32, tag="U")
            L = sw
            while L > 2:
                h = L // 2
                nc.vector.tensor_tensor(
                    out=Mflat[:, 0:h, :], in0=Mflat[:, 0:h, :],
                    in1=Mflat[:, L - h:L, :], op=mybir.AluOpType.add)
                L = L - h
            if L == 2:
                nc.vector.tensor_tensor(
                    out=U[:].unsqueeze(1), in0=Mflat[:, 0:1, :],
                    in1=Mflat[:, 1:2, :], op=mybir.AluOpType.add)
            else:
                nc.vector.tensor_copy(out=U[:].unsqueeze(1),
                                      in_=Mflat[:, 0:1, :])
            D = work.tile([128, H], f32, tag="D")
            nc.vector.tensor_reduce(out=D[:], in_=Ebf[:].transpose([0, 2, 1]),
                                    axis=mybir.AxisListType.X,
                                    op=mybir.AluOpType.add)
            Dinv = work.tile([128, H], f32, tag="Dinv")
            nc.vector.reciprocal(out=Dinv[:], in_=D[:])
            O = work.tile([128, HF], f32, tag="O")
            nc.vector.tensor_tensor(
                out=O[:].rearrange("p (h f) -> p h f", h=H),
                in0=U[:].rearrange("p (h f) -> p h f", h=H),
                in1=Dinv[:].unsqueeze(2).to_broadcast([128, H, F]),
                op=mybir.AluOpType.mult)
            nc.vector.tensor_tensor(out=O[:], in0=O[:], in1=sk_ps[:, 0:HF],
                                    op=mybir.AluOpType.add)
            # ELU = (relu(O) - 1) + exp(O - relu(O))
            R_t = work.tile([128, HF], f32, tag="Rt")
            nc.scalar.activation(out=R_t[:], in_=O[:],
                                 func=mybir.ActivationFunctionType.Relu)
            A_t = work.tile([128, HF], f32, tag="At")
            nc.scalar.activation(out=A_t[:], in_=R_t[:],
                                 func=mybir.ActivationFunctionType.Copy,
                                 bias=-1.0)
            T_t = work.tile([128, HF], f32, tag="Tt")
            nc.vector.tensor_tensor(out=T_t[:], in0=O[:], in1=R_t[:],
                                    op=mybir.AluOpType.subtract)
            E2 = work.tile([128, HF], f32, tag="E2")
            nc.scalar.activation(out=E2[:], in_=T_t[:],
                                 func=mybir.ActivationFunctionType.Exp)
            OUT = work.tile([128, HF], f32, tag="OUT")
            nc.vector.tensor_tensor(out=OUT[:], in0=A_t[:], in1=E2[:],
                                    op=mybir.AluOpType.add)
            nc.sync.dma_start(out=out_d[b * 128:(b + 1) * 128, :],
                              in_=OUT[:])

    nc.compile()
    return nc


def kernel(x, edge_index, W_proj, W_skip, a_src, a_tgt):
    common, per_core = _host_prep(x, edge_index, W_proj, W_skip, a_src, a_tgt)
    key = "prog"
    if key not in _COMPILED:
        _COMPILED[key] = _build_program(common["W"])
    nc = _COMPILED[key]

    in_maps = []
    for c in range(N_CORES):
        pc = per_core[c]
        in_maps.append({
            "xT": common["xT"],
            "xTperm": pc["xTperm"],
            "pack0": common["pack0"],
            "pack2": common["pack2"],
            "idxs": pc["idxs"],
            "gcnt": pc["gcnt"],
        })
    trace = bool(int(os.environ.get("GAT_TRACE", "0")))
    res = run_bass_kernel_spmd(nc, in_maps, list(range(N_CORES)),
                               trace=trace)
    if trace:
        kernel.last_exec_time_ns = res.exec_time_ns
        kernel.last_mean_exec_time_ns = res.mean_exec_time_ns

    out = np.empty((N_NODES, HF), np.float32)
    for c in range(N_CORES):
        o = res.results[c]["out"]  # [12544, 128] in rank order
        perm = per_core[c]["perm"]
        out[c * TGT_PER_CORE + perm] = o[:TGT_PER_CORE]
    return out


kernel.last_exec_time_ns = None
kernel.last_mean_exec_time_ns = None
